# revision 1
# baseline (speedup 1.0000x reference)
"""Trainium2 Bass kernel for LoRA self-attention (nn_LoRAAttnProcessor).

Problem shapes (hardcoded): x [2, 2048, 1280], 20 heads x 64 dim, LoRA rank 4.

Strategy
--------
* Host side: fold every LoRA pair into its base weight (W_eff = W + B @ A) --
  mathematically identical (associativity), and fold the 1/sqrt(D) score
  scale into Wq_eff.  The kernel then computes plain multi-head attention.
* Sharding: 8 cores x (batch b = core//4, 5 heads = core%4).  Wq/Wk/Wv are
  column-sharded by head, Wo row-sharded by head; each core emits a partial
  output [2048, 1280] that the host sums per batch element (+ bias bo).
* Per core (S=2048, C=1280, 5 local heads, D=64), all matmuls in bf16 with
  fp32 PSUM accumulation:
    A2: v   = x @ WvT_local            (x^T chunks stationary)   [S, 320]
    A1: qkT = Wqk_local @ x^T          (weight chunks stationary)[768, S]
    attention per (head, 1024-wide query block), k-major:
        scoresT[sk,128 x sq,1024] = kT^T qT   (K=64 contraction)
        probsT = exp(scoresT)               (one ACT op per psum tile)
        ctxT[65, sq] += [v_h | 1]^T probsT  (row 64 = softmax denominator)
    normalize: ctxT[0:64] * recip(row64) via DRAM-bounce broadcast
    out_part[m,128 x 1280] = ctxT^T @ WoT_local (accumulated over 3 j-chunks)
"""

import sys

if "/opt/trn_rl_repo" not in sys.path:
    sys.path.insert(0, "/opt/trn_rl_repo")

from contextlib import ExitStack

import ml_dtypes
import numpy as np

import concourse.bass as bass
import concourse.tile as tile
from concourse import bacc, mybir
from concourse.bass_utils import run_bass_kernel_spmd

BF16 = mybir.dt.bfloat16
F32 = mybir.dt.float32
NPBF16 = ml_dtypes.bfloat16

D = 64
H_LOC = 5  # heads per core
N_CORES = 8


def _q_loc(h):
    """(chunk, partition offset) of qT for local head h in qkT_sb."""
    return (h // 2, (h % 2) * 64) if h < 4 else (4, 0)


def _k_loc(h):
    return (2 + h // 2, (h % 2) * 64) if h < 4 else (5, 0)


def build_program(S=2048, C=1280, paired=False, interleave=False, repeat=1):
    # HW A/B (repeat-differential timing): unpaired+serial emission measured
    # fastest (~374us/body); row-group pairing and A1/attention interleaving
    # both regressed, so they stay off by default.
    """Build the SPMD single-core program. S % 512 == 0, C % 128 == 0."""
    assert S % 512 == 0 and C % 128 == 0
    CK = C // 128         # contraction chunks over channels
    SM = S // 128         # 128-row chunks of sequence
    SN = S // 512         # 512-col chunks of sequence
    SQB = min(1024, S)    # query block width (psum-limited)
    NSQ = S // SQB
    SK = S // 128         # key chunks
    NQ = SQB // 512

    nc = bacc.Bacc("TRN2", target_bir_lowering=False, debug=False)

    xT_d = nc.dram_tensor("xT", [C, S], BF16, kind="ExternalInput").ap()
    wqk_d = nc.dram_tensor("wqk", [C, 768], BF16, kind="ExternalInput").ap()
    wvT_d = nc.dram_tensor("wvT", [C, H_LOC * D], BF16, kind="ExternalInput").ap()
    woT_d = nc.dram_tensor("woT", [384, C], BF16, kind="ExternalInput").ap()
    out_d = nc.dram_tensor("out_part", [S, C], F32, kind="ExternalOutput").ap()

    EXP = mybir.ActivationFunctionType.Exp
    MULT = mybir.AluOpType.mult

    with tile.TileContext(nc) as tc, ExitStack() as ctx:
        persist = ctx.enter_context(tc.tile_pool(name="persist", bufs=1))
        psp = ctx.enter_context(tc.tile_pool(name="ps", bufs=4, space="PSUM"))
        ppool = ctx.enter_context(tc.tile_pool(name="probs", bufs=4))
        smallp = ctx.enter_context(tc.tile_pool(name="small", bufs=2))
        outp = ctx.enter_context(tc.tile_pool(name="osb", bufs=3))
        dramp = ctx.enter_context(tc.tile_pool(name="scratch", bufs=2, space="DRAM"))

        xT_sb = persist.tile([128, CK, S], BF16, tag="xT")
        wqk_sb = persist.tile([128, CK, 768], BF16, tag="wqk")
        wvT_sb = persist.tile([128, CK, H_LOC * D], BF16, tag="wvT")
        woT_sb = persist.tile([128, 3, C], BF16, tag="woT")
        qkT_sb = persist.tile([128, 6, S], BF16, tag="qkT")
        v_sb = persist.tile([128, SM, H_LOC, D + 1], BF16, tag="vsb")
        ctxT_sb = persist.tile([128, 3, S], BF16, tag="ctxT")

        def emit_body(rep):
            nc.sync.dma_start(xT_sb[:], xT_d.rearrange("(o p) n -> p o n", p=128))
            nc.sync.dma_start(wqk_sb[:], wqk_d.rearrange("(o p) n -> p o n", p=128))
            nc.sync.dma_start(wvT_sb[:], wvT_d.rearrange("(o p) n -> p o n", p=128))
            nc.sync.dma_start(woT_sb[:], woT_d.rearrange("(o p) n -> p o n", p=128))

            # ones column for the softmax-denominator trick; zero the 64 pad
            # partitions of the last ctxT chunk (head 4 has no pair).
            nc.vector.memset(v_sb[:, :, :, D : D + 1], 1.0)
            nc.vector.memset(ctxT_sb[64:128, 2, :], 0.0)

            # ---- A2: v = x @ WvT_local  -> v_sb[s-chunk, head, 0:64] ----
            def emit_a2():
                for m in range(SM):
                  ps = psp.tile([128, 1024], F32, tag="ps")
                  for c in range(CK):
                      nc.tensor.matmul(
                          ps[:, 0 : H_LOC * D],
                          lhsT=xT_sb[:, c, m * 128 : (m + 1) * 128],
                          rhs=wvT_sb[:, c, :],
                          start=(c == 0),
                          stop=(c == CK - 1),
                      )
                  nc.vector.tensor_copy(
                      out=v_sb[:, m, :, 0:D],
                      in_=ps[:, 0 : H_LOC * D].rearrange("p (h d) -> p h d", h=H_LOC),
                  )

            # ---- A1: qkT = Wqk^T @ xT  -> qkT_sb[f-chunk, s] ----
            # weight chunk stays stationary across up to 4 moving x slices
            def emit_a1(f):
                for s0 in range(0, SN, 4):
                    group = list(range(s0, min(s0 + 4, SN)))
                    tiles = {}
                    for gi, s in enumerate(group):
                        if gi % 2 == 0:
                            tiles[gi // 2] = psp.tile(
                                [128, 1024], F32, tag="ps", name=f"a1ps_{f}_{s0}_{gi}"
                            )
                    for c in range(CK):
                        for gi, s in enumerate(group):
                            pst = tiles[gi // 2]
                            off = (gi % 2) * 512
                            nc.tensor.matmul(
                                pst[:, off : off + 512],
                                lhsT=wqk_sb[:, c, f * 128 : (f + 1) * 128],
                                rhs=xT_sb[:, c, s * 512 : (s + 1) * 512],
                                start=(c == 0),
                                stop=(c == CK - 1),
                            )
                    for gi, s in enumerate(group):
                        if gi % 2 == 0:
                            w = min(1024, (len(group) - gi) * 512)
                            nc.vector.tensor_copy(
                                out=qkT_sb[:, f, s * 512 : s * 512 + w],
                                in_=tiles[gi // 2][:, 0:w],
                            )

            # ---- attention ----
            def emit_attention(heads):
                """heads: 1 (solo) or 2 (row-group-paired) local head indices."""
                locs = []
                for h in heads:
                    qc, qo = _q_loc(h)
                    kc, ko = _k_loc(h)
                    assert qo == ko and qo == (h % 2) * 64
                    locs.append((h, qc, kc, qo, h // 2, (h % 2) * 64))
                for sq in range(NSQ):
                    ctxs = {
                        h: psp.tile([128, 1024], F32, tag="ps", name=f"ctx_{h}_{sq}")
                        for h in heads
                    }
                    for sk in range(SK):
                        scs = {
                            h: psp.tile([128, 1024], F32, tag="ps", name=f"sc_{h}_{sq}_{sk}")
                            for h in heads
                        }
                        # paired heads sit in distinct PE row groups -> concurrent
                        for n in range(NQ):
                            for h, qc, kc, o, _, _ in locs:
                                nc.tensor.matmul(
                                    scs[h][:, n * 512 : (n + 1) * 512],
                                    lhsT=qkT_sb[o : o + D, kc, sk * 128 : (sk + 1) * 128],
                                    rhs=qkT_sb[
                                        o : o + D,
                                        qc,
                                        sq * SQB + n * 512 : sq * SQB + (n + 1) * 512,
                                    ],
                                    start=True,
                                    stop=True,
                                )
                        pts = {}
                        for h, *_ in locs:
                            pt = ppool.tile([128, SQB], BF16, tag="probs", name=f"pt_{h}")
                            nc.scalar.activation(pt[:, 0:SQB], scs[h][:, 0:SQB], EXP)
                            pts[h] = pt
                        for n in range(NQ):
                            for h, *_ in locs:
                                nc.tensor.matmul(
                                    ctxs[h][0 : D + 1, n * 512 : (n + 1) * 512],
                                    lhsT=v_sb[:, sk, h, :],
                                    rhs=pts[h][:, n * 512 : (n + 1) * 512],
                                    start=(sk == 0),
                                    stop=(sk == SK - 1),
                                )
                    # normalize: ctxT = ctx[0:64] * recip(ctx[64])
                    for h, qc, kc, o, jc, po in locs:
                        ctx_ps = ctxs[h]
                        rec = smallp.tile([1, SQB], F32, tag="rec", name=f"rec_{h}")
                        nc.vector.reciprocal(rec[:], ctx_ps[D : D + 1, 0:SQB])
                        scr = dramp.tile([1, SQB], F32, name=f"scr_{h}_{sq}")
                        nc.sync.dma_start(scr[:], rec[:])
                        bc = smallp.tile([64, SQB], F32, tag="bc", name=f"bc_{h}")
                        nc.sync.dma_start(bc[:], scr[:].to_broadcast((64, SQB)))
                        nc.vector.tensor_tensor(
                            out=ctxT_sb[po : po + D, jc, sq * SQB : (sq + 1) * SQB],
                            in0=ctx_ps[0:D, 0:SQB],
                            in1=bc[:],
                            op=MULT,
                        )

            # interleave A1 with attention: emit each pair's projection chunks
            # right before the attention that consumes them, so projection matmuls
            # fill PE idle slots of the ACT-bound previous attention phase.
            if paired:
                groups = [[0, 1], [2, 3], [4]]
            else:
                groups = [[0], [1], [2], [3], [4]]
            if interleave:
                a1_sched = {0: [0, 2], 1: [1, 3], 2: [4, 5]} if paired else {
                    0: [0, 2], 1: [1, 3], 2: [4, 5], 3: [], 4: []
                }
            else:
                a1_sched = {0: list(range(6))}
            first = True
            for gi, g in enumerate(groups):
                for f in a1_sched.get(gi, []):
                    emit_a1(f)
                if first:
                    emit_a2()
                    first = False
                emit_attention(g)

            # ---- output projection: out = ctxT^T @ WoT ----
            col_slices = []  # (col0, width, tile_idx, tile_off)
            acc_off, ti = 0, 0
            for col0 in range(0, C, 512):
                w = min(512, C - col0)
                if acc_off + w > 1024:
                    ti, acc_off = ti + 1, 0
                col_slices.append((col0, w, ti, acc_off))
                acc_off += w
            ntiles = ti + 1
            for m in range(SM):
                otiles = [
                    psp.tile([128, 1024], F32, tag="ps", name=f"ops_{m}_{t}")
                    for t in range(ntiles)
                ]
                for j in range(3):
                    lhsT = ctxT_sb[:, j, m * 128 : (m + 1) * 128]
                    for col0, w, t, toff in col_slices:
                        nc.tensor.matmul(
                            otiles[t][:, toff : toff + w],
                            lhsT=lhsT,
                            rhs=woT_sb[:, j, col0 : col0 + w],
                            start=(j == 0),
                            stop=(j == 2),
                        )
                out_sb = outp.tile([128, C], F32, tag="osb")
                for col0, w, t, toff in col_slices:
                    nc.vector.tensor_copy(
                        out=out_sb[:, col0 : col0 + w], in_=otiles[t][:, toff : toff + w]
                    )
                nc.sync.dma_start(out_d[m * 128 : (m + 1) * 128, :], out_sb[:])

        for rep in range(repeat):
            emit_body(rep)


    nc.compile()
    return nc


def make_core_inputs(x, Wq_eff, Wk_eff, Wv_eff, Wo_eff):
    """Per-core input dicts. x [B,S,C] f32; W_eff [C,C] f32 (scale folded)."""
    B, S, C = x.shape
    in_maps = []
    xT16 = [np.ascontiguousarray(x[b].T).astype(NPBF16) for b in range(B)]
    for core in range(N_CORES):
        b, g = core // 4, core % 4
        r0 = g * H_LOC * D  # first feature row of this core's heads
        qf = Wq_eff[r0 : r0 + H_LOC * D]  # (320, C)
        kf = Wk_eff[r0 : r0 + H_LOC * D]
        vf = Wv_eff[r0 : r0 + H_LOC * D]
        zero = np.zeros((D, C), np.float32)
        # chunks: (q0,q1)(q2,q3)(k0,k1)(k2,k3)(q4,0)(k4,0)
        wqk = np.concatenate(
            [qf[: 4 * D], kf[: 4 * D], qf[4 * D :], zero, kf[4 * D :], zero], axis=0
        ).T  # (C, 768)
        wvT = vf.T  # (C, 320)
        woT = np.concatenate(
            [Wo_eff[:, r0 : r0 + H_LOC * D].T, np.zeros((D, C), np.float32)], axis=0
        )  # (384, C)
        in_maps.append(
            {
                "xT": xT16[b],
                "wqk": np.ascontiguousarray(wqk).astype(NPBF16),
                "wvT": np.ascontiguousarray(wvT).astype(NPBF16),
                "woT": np.ascontiguousarray(woT).astype(NPBF16),
            }
        )
    return in_maps


def fold_weights(Wq, Wk, Wv, Wo, Aq, Bq, Ak, Bk, Av, Bv, Ao, Bo):
    scale = 1.0 / np.sqrt(np.float32(D))
    Wq_eff = (Wq + Bq @ Aq) * scale
    Wk_eff = Wk + Bk @ Ak
    Wv_eff = Wv + Bv @ Av
    Wo_eff = Wo + Bo @ Ao
    return Wq_eff, Wk_eff, Wv_eff, Wo_eff


_NC_CACHE = {}


def _get_program(S, C):
    key = (S, C)
    if key not in _NC_CACHE:
        _NC_CACHE[key] = build_program(S, C)
    return _NC_CACHE[key]


def kernel(**inputs):
    inputs = {k: np.asarray(v, np.float32) for k, v in inputs.items()}
    x = inputs["x"]
    B, S, C = x.shape
    Wq_eff, Wk_eff, Wv_eff, Wo_eff = fold_weights(
        inputs["Wq"], inputs["Wk"], inputs["Wv"], inputs["Wo"],
        inputs["Aq"], inputs["Bq"], inputs["Ak"], inputs["Bk"],
        inputs["Av"], inputs["Bv"], inputs["Ao"], inputs["Bo"],
    )
    in_maps = make_core_inputs(x, Wq_eff, Wk_eff, Wv_eff, Wo_eff)
    nc = _get_program(S, C)
    res = run_bass_kernel_spmd(nc, in_maps, list(range(N_CORES)))
    parts = [res.results[c]["out_part"].astype(np.float32) for c in range(N_CORES)]
    bo = inputs["bo"]
    out = np.stack(
        [
            parts[0] + parts[1] + parts[2] + parts[3] + bo,
            parts[4] + parts[5] + parts[6] + parts[7] + bo,
        ]
    ).astype(np.float32)
    return out



# revision 15
# speedup vs baseline: 1.1414x; 1.1414x over previous
"""Trainium2 Bass kernel for LoRA self-attention (nn_LoRAAttnProcessor).

Problem shapes (hardcoded): x [2, 2048, 1280], 20 heads x 64 dim, LoRA rank 4.

Strategy
--------
* Host side: fold every LoRA pair into its base weight (W_eff = W + B @ A) --
  mathematically identical (associativity), and fold the 1/sqrt(D) score
  scale into Wq_eff.  The kernel then computes plain multi-head attention.
* Sharding: 8 cores x (batch b = core//4, 5 heads = core%4).  Wq/Wk/Wv are
  column-sharded by head, Wo row-sharded by head; each core emits a partial
  TRANSPOSED output [1280, 2048] that the host sums per batch element and
  transposes (+ bias bo).
* Per core (S=2048, C=1280, 5 local heads, D=64), all matmuls in bf16 with
  fp32 PSUM accumulation.  Schedule: chunked input DMAs (host pre-shuffles
  weights/activations into partition-major blocks so every DMA moves large
  contiguous runs), software pipelining of the projection work (A1 qk-proj,
  A2 v-proj, transposed out-proj) into the attention phase, dedicated PSUM
  pools (scores x2 | ctx | bcast | aux = 8 banks exactly), matmul-based
  softmax-denominator broadcast, and output DMAs issued from the otherwise
  idle GPSIMD queue so they never serialize against next-iteration input
  DMAs on the SP queue.
"""

import sys

if "/opt/trn_rl_repo" not in sys.path:
    sys.path.insert(0, "/opt/trn_rl_repo")

from contextlib import ExitStack

import ml_dtypes
import numpy as np

import concourse.bass as bass
import concourse.tile as tile
from concourse import bacc, mybir
from concourse.bass_utils import run_bass_kernel_spmd

BF16 = mybir.dt.bfloat16
F32 = mybir.dt.float32
NPBF16 = ml_dtypes.bfloat16

D = 64
H_LOC = 5  # heads per core
N_CORES = 8


def _q_loc(h):
    """(chunk, partition offset) of qT for local head h in qkT_sb."""
    return (h // 2, (h % 2) * 64) if h < 4 else (4, 0)


def _k_loc(h):
    return (2 + h // 2, (h % 2) * 64) if h < 4 else (5, 0)


def build_program(S=2048, C=1280, paired=False, interleave=False, repeat=1,
                  flip=False):
    """Build the SPMD single-core program. S == 2048, C == 1280.

    flip=False: ctx accumulated k-major as ctxT [65, q] (2 fat matmuls/step),
    softmax denominator broadcast via ones-matmul.
    flip=True: ctx accumulated q-major as [q, 65] (8 thin matmuls/step, half
    the PE stream cycles), per-partition normalize, PE-transpose back to ctxT.
    """
    assert S == 2048 and C % 128 == 0
    CK = C // 128         # contraction chunks over channels (10)
    SN = S // 512         # 512-col slices of sequence (4)
    SQB = 1024            # query block width
    NSQ = S // SQB        # 2
    SK = S // 128         # key chunks (16)
    NQ = SQB // 512       # 2
    CCH = C // 128        # out-proj column chunks (10)

    nc = bacc.Bacc("TRN2", target_bir_lowering=False, debug=False)

    # Host pre-shuffled, partition-major inputs (big contiguous DMA runs).
    xT_d = nc.dram_tensor("xT", [128, SN, CK, 512], BF16, kind="ExternalInput").ap()
    wqk_d = nc.dram_tensor("wqk", [128, 6, CK, 128], BF16, kind="ExternalInput").ap()
    wvT_d = nc.dram_tensor(
        "wvT", [128, CK, H_LOC * D], BF16, kind="ExternalInput"
    ).ap()
    woT_d = nc.dram_tensor("woT", [128, 3, C], BF16, kind="ExternalInput").ap()
    if flip:
        ident_d = nc.dram_tensor("ident", [128, 128], BF16, kind="ExternalInput").ap()
    out_d = nc.dram_tensor("outT_part", [C, S], F32, kind="ExternalOutput").ap()

    EXP = mybir.ActivationFunctionType.Exp
    MULT = mybir.AluOpType.mult

    with tile.TileContext(nc) as tc, ExitStack() as ctx:
        persist = ctx.enter_context(tc.tile_pool(name="persist", bufs=1))
        # PSUM budget (8 banks of 2KB):
        #   spool 2 x [128,1024]f32 = 4 | cpool [*,1024]f32 = 2
        #   bcpool/psT = 1             | aux [128,512]f32 = 1
        spool = ctx.enter_context(tc.tile_pool(name="sc", bufs=2, space="PSUM"))
        cpool = ctx.enter_context(tc.tile_pool(name="cx", bufs=1, space="PSUM"))
        bcpool = ctx.enter_context(tc.tile_pool(name="bc", bufs=1, space="PSUM"))
        aux = ctx.enter_context(tc.tile_pool(name="aux", bufs=1, space="PSUM"))
        ppool = ctx.enter_context(tc.tile_pool(name="probs", bufs=4))
        smallp = ctx.enter_context(tc.tile_pool(name="small", bufs=2))
        outp = ctx.enter_context(tc.tile_pool(name="osb", bufs=8))

        xT_sb = persist.tile([128, SN, CK, 512], BF16, tag="xT")
        wqk_sb = persist.tile([128, 6, CK, 128], BF16, tag="wqk")
        wvT_sb = persist.tile([128, CK, H_LOC * D], BF16, tag="wvT")
        woT_sb = persist.tile([128, 3, C], BF16, tag="woT")
        qkT_sb = persist.tile([128, 6, S], BF16, tag="qkT")
        v_sb = persist.tile([128, SK, H_LOC, D + 1], BF16, tag="vsb")
        ctxT_sb = persist.tile([128, 3, S], BF16, tag="ctxT")
        ones_sb = persist.tile([1, D], BF16, tag="ones")
        if flip:
            ident_sb = persist.tile([128, 128], BF16, tag="ident")

        def emit_body(rep):
            # ---- chunked input DMAs, in consumption order ----
            def dma_wqk(f):
                nc.sync.dma_start(wqk_sb[:, f], wqk_d[:, f])

            def dma_xt(s):
                nc.sync.dma_start(xT_sb[:, s], xT_d[:, s])

            dma_wqk(0)
            dma_xt(0)
            dma_wqk(2)
            nc.sync.dma_start(wvT_sb[:], wvT_d[:])
            dma_xt(1)
            dma_wqk(1)
            dma_wqk(3)
            dma_xt(2)
            dma_xt(3)
            dma_wqk(4)
            dma_wqk(5)
            nc.sync.dma_start(woT_sb[:], woT_d[:])
            if flip:
                nc.sync.dma_start(ident_sb[:], ident_d[:])

            # ones column for the softmax-denominator trick; zero the 64 pad
            # partitions of the last ctxT chunk (head 4 has no pair); ones row
            # for the denominator-broadcast matmul.
            nc.vector.memset(v_sb[:, :, :, D : D + 1], 1.0)
            nc.vector.memset(ctxT_sb[64:128, 2, :], 0.0)
            nc.vector.memset(ones_sb[:], 1.0)

            # ---- unit emitters ----
            def a1(f, s, pool):
                """qkT chunk: [128 feat, 512 seq] = wqk_f^T @ xT_slice."""
                ps = pool.tile([128, 512], F32, tag="sc" if pool is spool else "aux",
                               name=f"a1ps_{rep}_{f}_{s}")
                for c in range(CK):
                    nc.tensor.matmul(
                        ps[:, 0:512],
                        lhsT=wqk_sb[:, f, c, :],
                        rhs=xT_sb[:, s, c, :],
                        start=(c == 0),
                        stop=(c == CK - 1),
                    )
                nc.vector.tensor_copy(
                    out=qkT_sb[:, f, s * 512 : (s + 1) * 512], in_=ps[:, 0:512]
                )

            def a2(m, pool):
                """v chunk: [128 seq, 320 feat] = xT_m^T @ wvT."""
                ps = pool.tile([128, 512], F32, tag="sc" if pool is spool else "aux",
                               name=f"a2ps_{rep}_{m}")
                s, j = m // 4, (m % 4) * 128
                for c in range(CK):
                    nc.tensor.matmul(
                        ps[:, 0 : H_LOC * D],
                        lhsT=xT_sb[:, s, c, j : j + 128],
                        rhs=wvT_sb[:, c, :],
                        start=(c == 0),
                        stop=(c == CK - 1),
                    )
                nc.vector.tensor_copy(
                    out=v_sb[:, m, :, 0:D],
                    in_=ps[:, 0 : H_LOC * D].rearrange("p (h d) -> p h d", h=H_LOC),
                )

            def op(cc, qb, pool, ptag):
                """outT block: [128 cols, 512 seq] = Wo_loc^T-chunk @ ctxT."""
                ps = pool.tile([128, 512], F32, tag=ptag, name=f"ops_{rep}_{cc}_{qb}")
                for j in range(3):
                    nc.tensor.matmul(
                        ps[:, 0:512],
                        lhsT=woT_sb[:, j, cc * 128 : (cc + 1) * 128],
                        rhs=ctxT_sb[:, j, qb * 512 : (qb + 1) * 512],
                        start=(j == 0),
                        stop=(j == 2),
                    )
                ob = outp.tile([128, 512], F32, tag="osb", name=f"ob_{rep}_{cc}_{qb}")
                nc.vector.tensor_copy(out=ob[:], in_=ps[:, 0:512])
                nc.gpsimd.dma_start(
                    out_d[cc * 128 : (cc + 1) * 128, qb * 512 : (qb + 1) * 512], ob[:]
                )

            # ---- attention units ----
            def scores_unit(h, sq, sk):
                """scoresT psum [128 k, 1024 q] + exp -> probs bf16."""
                qc, qo = _q_loc(h)
                kc, ko = _k_loc(h)
                sc = spool.tile([128, SQB], F32, tag="sc", name=f"sc_{h}_{sq}_{sk}")
                for n in range(NQ):
                    nc.tensor.matmul(
                        sc[:, n * 512 : (n + 1) * 512],
                        lhsT=qkT_sb[ko : ko + D, kc, sk * 128 : (sk + 1) * 128],
                        rhs=qkT_sb[
                            qo : qo + D,
                            qc,
                            sq * SQB + n * 512 : sq * SQB + (n + 1) * 512,
                        ],
                        start=True,
                        stop=True,
                    )
                pt = ppool.tile([128, SQB], BF16, tag="probs", name=f"pt_{h}_{sq}_{sk}")
                nc.scalar.activation(pt[:, 0:SQB], sc[:, 0:SQB], EXP)
                return pt

            def ctx_unit(h, sq, sk, pt, ctx_ps):
                if flip:
                    # PSUM start zeroes a whole 2KB bank (lazy pending-zero):
                    # only the first qc group of each bank may issue start, the
                    # other three inherit the bank's pending-zero state.
                    for qc in range(SQB // 128):
                        nc.tensor.matmul(
                            ctx_ps[:, qc, 0 : D + 1],
                            lhsT=pt[:, qc * 128 : (qc + 1) * 128],
                            rhs=v_sb[:, sk, h, :],
                            start=(sk == 0 and qc % 4 == 0),
                            stop=(sk == SK - 1 and qc % 4 == 0),
                            skip_group_check=True,
                        )
                else:
                    for n in range(NQ):
                        nc.tensor.matmul(
                            ctx_ps[0 : D + 1, n * 512 : (n + 1) * 512],
                            lhsT=v_sb[:, sk, h, :],
                            rhs=pt[:, n * 512 : (n + 1) * 512],
                            start=(sk == 0),
                            stop=(sk == SK - 1),
                        )

            def nrm(h, sq, ctx_ps):
                """Normalize + store ctxT bf16 for the out-projection."""
                jc, po = h // 2, (h % 2) * 64
                if flip:
                    # per-partition softmax normalize, then PE-transpose back
                    rec = smallp.tile([128, 8], F32, tag="recf", name=f"rec_{h}_{sq}")
                    nc.vector.reciprocal(rec[:], ctx_ps[:, :, D])
                    ctxn = smallp.tile(
                        [128, 8, D], BF16, tag="ctxn", name=f"ctxn_{h}_{sq}"
                    )
                    for qc in range(8):
                        nc.vector.tensor_scalar_mul(
                            ctxn[:, qc, :], ctx_ps[:, qc, 0:D], rec[:, qc : qc + 1]
                        )
                    psT = _flip_state.get("psT")
                    if psT is None:
                        psT = bcpool.tile(
                            [128, SQB], BF16, tag="bc", name=f"psT_{h}_{sq}"
                        )
                        _flip_state["psT"] = psT
                    for qc in range(8):
                        nc.tensor.transpose(
                            psT[po : po + D, qc * 128 : (qc + 1) * 128],
                            ctxn[:, qc, :],
                            ident_sb[:],
                        )
                    if h % 2 == 1 or h == 4:
                        rows = 128 if h % 2 == 1 else 64
                        nc.vector.tensor_copy(
                            out=ctxT_sb[0:rows, jc, sq * SQB : (sq + 1) * SQB],
                            in_=psT[0:rows, :],
                        )
                        _flip_state["psT"] = None
                else:
                    rec = smallp.tile([1, SQB], BF16, tag="rec", name=f"rec_{h}_{sq}")
                    with nc.allow_low_precision(reason="softmax recip bf16"):
                        nc.vector.reciprocal(rec[:], ctx_ps[D : D + 1, 0:SQB])
                    for half in range(2):
                        c0 = half * 512
                        bc = bcpool.tile(
                            [D, 512], F32, tag="bc", name=f"bc_{h}_{sq}_{half}"
                        )
                        nc.tensor.matmul(
                            bc[:, 0:512],
                            lhsT=ones_sb[0:1, :],
                            rhs=rec[0:1, c0 : c0 + 512],
                            start=True,
                            stop=True,
                        )
                        # DVE cannot read two PSUM operands; stage bc in SBUF
                        bcs = smallp.tile(
                            [D, 512], BF16, tag="bcs", name=f"bcs_{h}_{sq}_{half}"
                        )
                        nc.vector.tensor_copy(out=bcs[:], in_=bc[:, 0:512])
                        nc.vector.tensor_tensor(
                            out=ctxT_sb[
                                po : po + D, jc, sq * SQB + c0 : sq * SQB + c0 + 512
                            ],
                            in0=ctx_ps[0:D, c0 : c0 + 512],
                            in1=bcs[:],
                            op=MULT,
                        )

            _flip_state = {"psT": None}

            # ---- lead-in: enough projection work to start attention ----
            a1(0, 0, spool)
            a1(0, 1, spool)
            a1(2, 0, spool)
            a2(0, spool)
            a2(1, spool)

            # ---- filler schedule: (sq, h) -> 16 per-iteration buckets.
            # Ordering constraints: scores(h, sq, sk) needs k-chunk slice
            # floor(sk/4) of the head's k block BEFORE iteration sk, and
            # ctx(h, sq, m) (emitted at iteration m+1, before fillers) needs
            # a2(m) emitted at iteration <= m.
            def A1(f, s):
                return lambda: a1(f, s, aux)

            def A2(m):
                return lambda: a2(m, aux)

            def OP(c, q):
                return lambda: op(c, q, aux, "aux")

            def spread(fillers, step=2, start=0):
                buckets = [[] for _ in range(SK)]
                for i, f in enumerate(fillers):
                    buckets[min(start + i * step, SK - 1)].append(f)
                return buckets

            fill = {
                (0, 0): [
                    [A1(2, 1), A2(2)], [A2(3)], [A2(4)],
                    [A1(2, 2), A2(5)], [A2(6)], [A2(7)],
                    [A1(2, 3), A2(8)], [A2(9)], [A2(10)], [A2(11)], [A2(12)],
                    [A2(13)], [A2(14)], [A2(15)], [], [],
                ],
                (0, 1): spread(
                    [A1(1, 0), A1(3, 0), A1(1, 1), A1(3, 1), A1(3, 2), A1(3, 3)]
                ),
                (0, 2): spread(
                    [A1(4, 0), A1(5, 0), A1(4, 1), A1(5, 1), A1(5, 2), A1(5, 3)]
                ),
                (0, 3): spread([A1(0, 2), A1(0, 3), A1(1, 2), A1(1, 3)]),
                (0, 4): spread([A1(4, 2), A1(4, 3)]),
                # start=2: sq0's ctxT is only complete after the deferred
                # nrm of (0,4), which is emitted at iteration sk==1 here
                (1, 0): spread(
                    [OP(c, q) for c in range(4) for q in range(2)], start=2
                ),
                (1, 1): spread([OP(c, q) for c in range(4, 8) for q in range(2)]),
                (1, 2): spread([OP(c, q) for c in range(8, 10) for q in range(2)]),
                (1, 3): [[] for _ in range(SK)],
                (1, 4): [[] for _ in range(SK)],
            }

            # ---- main attention stream (sq-outer), software-pipelined ----
            prev_nrm = None  # deferred normalize of the previous head
            ctx_shape = [128, 8, 128] if flip else [128, SQB]
            for sq in range(NSQ):
                for h in range(H_LOC):
                    buckets = fill[(sq, h)]
                    # ctx tile is allocated lazily AFTER the previous head's
                    # deferred normalize is emitted: cpool has bufs=1, so the
                    # slot must have all its readers emitted before reuse.
                    ctx_ps = None
                    pts = {}
                    for sk in range(SK):
                        pts[sk] = scores_unit(h, sq, sk)
                        if sk == 1 and prev_nrm is not None:
                            # normalize the previous head once this head's
                            # first scores are in flight (frees its ctx tile
                            # well before our first ctx matmul needs it)
                            prev_nrm()
                            prev_nrm = None
                        if sk > 0:
                            if ctx_ps is None:
                                ctx_ps = cpool.tile(
                                    ctx_shape, F32, tag="ctx", name=f"ctx_{h}_{sq}"
                                )
                            ctx_unit(h, sq, sk - 1, pts.pop(sk - 1), ctx_ps)
                        for f in buckets[sk]:
                            f()
                    ctx_unit(h, sq, SK - 1, pts.pop(SK - 1), ctx_ps)
                    prev_nrm = (lambda hh, ss, cp: lambda: nrm(hh, ss, cp))(
                        h, sq, ctx_ps
                    )
            prev_nrm()

            # ---- tail: remaining out-proj blocks, rotating over freed pools ----
            pools = [(spool, "sc"), (cpool, "ctx"), (aux, "aux"), (spool, "sc")]
            i = 0
            for cc in range(CCH):
                for qb in (2, 3):
                    p, t = pools[i % len(pools)]
                    op(cc, qb, p, t)
                    i += 1

        for rep in range(repeat):
            emit_body(rep)

    nc.compile()
    return nc


def make_core_inputs(x, Wq_eff, Wk_eff, Wv_eff, Wo_eff):
    """Per-core input dicts. x [B,S,C] f32; W_eff [C,C] f32 (scale folded).

    All tensors are pre-shuffled into the kernel's partition-major DMA
    layouts so every chunk DMA moves large contiguous runs.
    """
    B, S, C = x.shape
    CK = C // 128
    in_maps = []
    xT_p = []
    for b in range(B):
        xT = np.ascontiguousarray(x[b].T).astype(NPBF16)  # [C, S]
        xT_p.append(
            np.ascontiguousarray(
                xT.reshape(CK, 128, 4, 512).transpose(1, 2, 0, 3)
            )
        )  # [128, 4, CK, 512]
    ident = np.eye(128, dtype=np.float32).astype(NPBF16)
    for core in range(N_CORES):
        b, g = core // 4, core % 4
        r0 = g * H_LOC * D  # first feature row of this core's heads
        qf = Wq_eff[r0 : r0 + H_LOC * D]  # (320, C)
        kf = Wk_eff[r0 : r0 + H_LOC * D]
        vf = Wv_eff[r0 : r0 + H_LOC * D]
        zero = np.zeros((D, C), np.float32)
        # chunks: (q0,q1)(q2,q3)(k0,k1)(k2,k3)(q4,0)(k4,0)
        wqk = np.concatenate(
            [qf[: 4 * D], kf[: 4 * D], qf[4 * D :], zero, kf[4 * D :], zero], axis=0
        ).T  # (C, 768)
        wqk_p = (
            wqk.astype(NPBF16).reshape(CK, 128, 6, 128).transpose(1, 2, 0, 3)
        )  # [128, 6, CK, 128]
        wvT_p = (
            vf.T.astype(NPBF16).reshape(CK, 128, H_LOC * D).transpose(1, 0, 2)
        )  # [128, CK, 320]
        woT = np.concatenate(
            [Wo_eff[:, r0 : r0 + H_LOC * D].T, np.zeros((D, C), np.float32)], axis=0
        )  # (384, C)
        woT_p = woT.astype(NPBF16).reshape(3, 128, C).transpose(1, 0, 2)  # [128,3,C]
        in_maps.append(
            {
                "xT": xT_p[b],
                "wqk": np.ascontiguousarray(wqk_p),
                "wvT": np.ascontiguousarray(wvT_p),
                "woT": np.ascontiguousarray(woT_p),
                "ident": ident,
            }
        )
    return in_maps


def fold_weights(Wq, Wk, Wv, Wo, Aq, Bq, Ak, Bk, Av, Bv, Ao, Bo):
    scale = 1.0 / np.sqrt(np.float32(D))
    Wq_eff = (Wq + Bq @ Aq) * scale
    Wk_eff = Wk + Bk @ Ak
    Wv_eff = Wv + Bv @ Av
    Wo_eff = Wo + Bo @ Ao
    return Wq_eff, Wk_eff, Wv_eff, Wo_eff


_NC_CACHE = {}


def _get_program(S, C):
    key = (S, C)
    if key not in _NC_CACHE:
        _NC_CACHE[key] = build_program(S, C)
    return _NC_CACHE[key]


def kernel(**inputs):
    inputs = {k: np.asarray(v, np.float32) for k, v in inputs.items()}
    x = inputs["x"]
    B, S, C = x.shape
    Wq_eff, Wk_eff, Wv_eff, Wo_eff = fold_weights(
        inputs["Wq"], inputs["Wk"], inputs["Wv"], inputs["Wo"],
        inputs["Aq"], inputs["Bq"], inputs["Ak"], inputs["Bk"],
        inputs["Av"], inputs["Bv"], inputs["Ao"], inputs["Bo"],
    )
    in_maps = make_core_inputs(x, Wq_eff, Wk_eff, Wv_eff, Wo_eff)
    nc = _get_program(S, C)
    for m in in_maps:
        m.pop("ident", None)  # non-flip program has no ident input
    res = run_bass_kernel_spmd(nc, in_maps, list(range(N_CORES)))
    parts = [res.results[c]["outT_part"].astype(np.float32) for c in range(N_CORES)]
    bo = inputs["bo"]
    out = np.stack(
        [
            (parts[0] + parts[1] + parts[2] + parts[3]).T + bo,
            (parts[4] + parts[5] + parts[6] + parts[7]).T + bo,
        ]
    ).astype(np.float32)
    return out
